# revision 7
# baseline (speedup 1.0000x reference)
"""Trainium2 Bass kernel for nn_CrossAttention_8495445311783.

Computation (per reference.py):
  q = wq@x+bq (1x1 conv, scaled), k = wk@ctx+bk, v = wv@ctx+bv
  attn = softmax((q*scale) @ k^T) over last dim, batched per (b, c, d)
  out = attn @ v + x

Sharding: (b, d) flattened to 256 slices, 32 per core (16 adjacent d-pairs).
Each core computes its slices fully independently (no collectives).

Layout strategy per d-pair (d0 = even, d1 = odd):
  - xin  [64,  8192] bf16  c-major:       free = dpar*4096 + h*64 + w
  - ctxin[128, 8192] bf16  c-major:       same free layout
  - QK_sb[128, 8192] bf16  rows 0:64 = q channels, 64:128 = k channels,
                           free = h*128 + dpar*64 + w   ("F layout")
  - QKT  [128, 8192] bf16  partitions = dpar*64 + w,
                           free = h*128 + (q c | k c + 64)  (PE transpose of QK)
  - V_sb [128, 4160] bf16  partitions = dpar*64 + g, free = c*65 + w,
                           ones column at c*65+64 (for softmax denominator)
  - attention per (c, dpar): mm1 lhsT=K^T[w,g], rhs=Q^T[w,h] -> AT[g,h] psum
    (d0 at psum rows 0:64, d1 at rows 64:128 via tile_position auto-derive);
    exp on [128,512] blocks; mm2 lhsT=P^T[g,h], rhs=[V|1][g,65] -> [h, w+denom]
  - normalize: TT multiply by reciprocal(denom) broadcast (stride-0 AP)
  - residual: SWDGE DMA-accumulate of x (f32) into out_sb, then DMA out.
"""

import sys

if "/opt/trn_rl_repo" not in sys.path:
    sys.path.insert(0, "/opt/trn_rl_repo")

import numpy as np
import ml_dtypes

B, CIN, CCTX, DIM, D, H, W = 4, 64, 128, 64, 64, 64, 64
S = H * W  # 4096
NCORES = 8
BD = B * D  # 256
SLICES_PER_CORE = BD // NCORES  # 32
NPAIRS = SLICES_PER_CORE // 2  # 16
SCALE = DIM ** -0.5

F32 = np.float32
BF16 = ml_dtypes.bfloat16


def _copy(nc, use_scalar, out, in_):
    if use_scalar:
        nc.scalar.copy(out, in_)
    else:
        nc.vector.tensor_copy(out, in_)


def build_kernel_body(nc, tc, tile, mybir, aps, n_pairs):
    """Emit the per-core kernel body inside TileContext tc.

    aps: dict with dram APs: xs [2P,64,4096] f32, cs [2P,128,4096] f32,
         wqT [64,64] bf16, wkT [128,64] bf16, wvT [128,64] bf16,
         bqk [1,128] bf16, bvp [1,512] bf16, out [2P,64,4096] f32.
    """
    from concourse.masks import make_identity

    dt = mybir.dt
    AF = mybir.ActivationFunctionType
    OP = mybir.AluOpType

    xs, cs, out = aps["xs"], aps["cs"], aps["out"]

    # ---- persistent constants in SBUF ----
    wq_sb = nc.alloc_sbuf_tensor("wq_sb", [CIN, DIM], dt.bfloat16).ap()
    wk_sb = nc.alloc_sbuf_tensor("wk_sb", [CCTX, DIM], dt.bfloat16).ap()
    wv_sb = nc.alloc_sbuf_tensor("wv_sb", [CCTX, DIM], dt.bfloat16).ap()
    bqk_sb = nc.alloc_sbuf_tensor("bqk_sb", [1, 128], dt.bfloat16).ap()
    bvp_sb = nc.alloc_sbuf_tensor("bvp_sb", [1, 512], dt.bfloat16).ap()
    ones_sb = nc.alloc_sbuf_tensor("ones_sb", [1, 512], dt.bfloat16).ap()
    id_sb = nc.alloc_sbuf_tensor("id_sb", [128, 128], dt.bfloat16).ap()

    nc.sync.dma_start(out=wq_sb, in_=aps["wqT"])
    nc.sync.dma_start(out=wk_sb, in_=aps["wkT"])
    nc.sync.dma_start(out=wv_sb, in_=aps["wvT"])
    nc.sync.dma_start(out=bqk_sb, in_=aps["bqk"])
    nc.sync.dma_start(out=bvp_sb, in_=aps["bvp"])
    nc.vector.memset(ones_sb, 1.0)
    make_identity(nc, id_sb)

    # V with ones columns must persist across iterations -> two explicit
    # buffers with the ones column initialized once.
    v_bufs = [
        nc.alloc_sbuf_tensor(f"v_sb{i}", [128, 65 * 64], dt.bfloat16).ap()
        for i in range(2)
    ]
    for vb in v_bufs:
        nc.vector.memset(
            vb.rearrange("p (c x) -> p c x", x=65)[:, :, 64:65], 1.0
        )

    with (
        tc.tile_pool(name="xin", bufs=2) as xin_pool,
        tc.tile_pool(name="ctxin", bufs=2) as ctx_pool,
        tc.tile_pool(name="qk", bufs=2) as qk_pool,
        tc.tile_pool(name="qkt", bufs=2) as qkt_pool,
        tc.tile_pool(name="osb", bufs=2) as osb_pool,
        tc.tile_pool(name="psb", bufs=3) as p_pool,
        tc.tile_pool(name="rc", bufs=8) as rc_pool,
        tc.tile_pool(name="psum", bufs=3, space="PSUM") as proj_psum,
        tc.tile_pool(name="psum_at", bufs=2, space="PSUM") as at_psum,
        tc.tile_pool(name="psum_o", bufs=3, space="PSUM") as o_psum,
    ):
        for p in range(n_pairs):
            v_sb = v_bufs[p % 2]

            xin = xin_pool.tile([CIN, 2 * S], dt.bfloat16, tag="xin")
            ctxin = ctx_pool.tile([CCTX, 2 * S], dt.bfloat16, tag="ctxin")
            qk_sb = qk_pool.tile([128, 2 * S], dt.bfloat16, tag="qk")
            qkt_sb = qkt_pool.tile([128, 2 * S], dt.bfloat16, tag="qkt")
            out_sb = osb_pool.tile([128, S], dt.float32, tag="osb")

            # ---- loads (SWDGE casts f32 -> bf16) ----
            nc.gpsimd.dma_start(
                out=xin[:].rearrange("c (d s) -> c d s", d=2),
                in_=xs[2 * p : 2 * p + 2].rearrange("d c s -> c d s"),
            )
            nc.gpsimd.dma_start(
                out=ctxin[:].rearrange("c (d s) -> c d s", d=2),
                in_=cs[2 * p : 2 * p + 2].rearrange("d c s -> c d s"),
            )

            # ---- q/k projection: 8 chunks of 512 cols in F order ----
            # rhs cols ordered (h, dpar, w): AP dims (4h, 2d, 64w) over
            # xin free = d*4096 + h*64 + w.
            xin_v = xin[:].rearrange("c (d h w) -> c h d w", d=2, w=W)
            ctx_v = ctxin[:].rearrange("c (d h w) -> c h d w", d=2, w=W)
            for ch in range(16):
                ps = proj_psum.tile([128, 512], dt.float32, tag="proj")
                h0 = 4 * ch
                # bias first: start=True zeroes the whole bank (2KB zero
                # region), writes bqk[m] everywhere; q/k then accumulate.
                nc.tensor.matmul(
                    ps[:, :],
                    bqk_sb,
                    ones_sb,
                    start=True,
                    stop=False,
                    skip_group_check=True,
                )
                nc.tensor.matmul(
                    ps[0:64, :].rearrange("p (h d w) -> p h d w", h=4, d=2),
                    wq_sb,
                    xin_v[:, h0 : h0 + 4],
                    start=False,
                    stop=False,
                    skip_group_check=True,
                )
                nc.tensor.matmul(
                    ps[64:128, :].rearrange("p (h d w) -> p h d w", h=4, d=2),
                    wk_sb,
                    ctx_v[:, h0 : h0 + 4],
                    start=False,
                    stop=True,
                    skip_group_check=True,
                )
                _copy(nc, ch % 2 == 0, qk_sb[:, ch * 512 : (ch + 1) * 512], ps[:, :])

            # ---- v projection (data-stationary): psum rows = dpar*64+g ----
            for wb in range(8):
                pv = proj_psum.tile([128, 512], dt.float32, tag="proj")
                # bias first: psum[m, (j c)] = bv[c]
                nc.tensor.matmul(
                    pv[:, :],
                    ones_sb[:, 0:128],
                    bvp_sb,
                    start=True,
                    stop=False,
                    skip_group_check=True,
                )
                for j in range(8):
                    w = wb * 8 + j
                    # lhsT: [c', 128] cols ordered (dpar, g), from ctxin
                    lhsT = ctxin[:].rearrange(
                        "c (d g w) -> c w d g", d=2, g=H
                    )[:, w]
                    nc.tensor.matmul(
                        pv[:, j * 64 : (j + 1) * 64],
                        lhsT,
                        wv_sb,
                        start=False,
                        stop=(j == 7),
                        skip_group_check=True,
                    )
                # scatter to V_sb cols c*65 + (wb*8+j)
                dst = v_sb.rearrange("p (c x) -> p c x", x=65)[
                    :, :, wb * 8 : wb * 8 + 8
                ].transpose([0, 2, 1])
                _copy(nc, wb % 2 == 0, dst, pv[:, :].rearrange("p (j c) -> p j c", j=8))

            # ---- transposes: QK [128, h-block 128] -> QKT ----
            for tb in range(8):
                pt = proj_psum.tile([128, 1024], dt.bfloat16, tag="proj")
                for t in range(8):
                    h = tb * 8 + t
                    nc.tensor.transpose(
                        pt[:, t * 128 : (t + 1) * 128],
                        qk_sb[:, h * 128 : (h + 1) * 128],
                        id_sb,
                    )
                _copy(nc, tb % 2 == 0, qkt_sb[:, tb * 1024 : (tb + 1) * 1024], pt[:, :])

            # ---- attention ----
            qkt_v = qkt_sb[:].rearrange("p (h c) -> p h c", c=128)
            for cg in range(8):
                pa = at_psum.tile([128, 512], dt.float32, tag="at")
                for cc in range(8):
                    c = cg * 8 + cc
                    for dpar in range(2):
                        b0 = dpar * 64
                        nc.tensor.matmul(
                            pa[b0 : b0 + 64, cc * 64 : (cc + 1) * 64],
                            qkt_v[b0 : b0 + 64, :, 64 + c],  # K^T [w, g]
                            qkt_v[b0 : b0 + 64, :, c],  # Q^T [w, h]
                            start=True,
                            stop=True,
                            skip_group_check=True,
                        )
                psb = p_pool.tile([128, 512], dt.bfloat16, tag="psb")
                nc.scalar.activation(psb[:, :], pa[:, :], AF.Exp)

                for half in range(2):
                    po = o_psum.tile([128, 512], dt.float32, tag="o")
                    for c4 in range(4):
                        cc = half * 4 + c4
                        c = cg * 8 + cc
                        for dpar in range(2):
                            b0 = dpar * 64
                            nc.tensor.matmul(
                                po[b0 : b0 + 64, c4 * 65 : (c4 + 1) * 65],
                                psb[b0 : b0 + 64, cc * 64 : (cc + 1) * 64],
                                v_sb[b0 : b0 + 64, c * 65 : c * 65 + 65],
                                start=True,
                                stop=True,
                                skip_group_check=True,
                            )
                    po_v = po[:, 0:260].rearrange("p (c x) -> p c x", x=65)
                    rc = rc_pool.tile([128, 4], dt.float32, tag="rc")
                    nc.vector.reciprocal(rc[:, :], po_v[:, :, 64])
                    # normalize: out = po[:, :, 0:64] * recip (broadcast)
                    c0 = (cg * 8 + half * 4) * 64
                    nc.vector.tensor_tensor(
                        out_sb[:, c0 : c0 + 256].rearrange(
                            "p (c w) -> p c w", w=64
                        ),
                        po_v[:, :, 0:64],
                        rc[:, :].unsqueeze(-1).broadcast_to([128, 4, 64]),
                        op=OP.mult,
                    )

            # ---- residual: accumulate x (f32) straight from HBM ----
            for dpar in range(2):
                b0 = dpar * 64
                nc.gpsimd.dma_start(
                    out=out_sb[b0 : b0 + 64, :].rearrange(
                        "h (c w) -> h c w", w=64
                    ),
                    in_=xs[2 * p + dpar].rearrange("c (h w) -> h c w", w=64),
                    accum_op=OP.add,
                )
                nc.sync.dma_start(
                    out=out[2 * p + dpar].rearrange("c (h w) -> h c w", w=64),
                    in_=out_sb[b0 : b0 + 64, :].rearrange(
                        "h (c w) -> h c w", w=64
                    ),
                )


def build_nc(n_pairs=NPAIRS):
    import concourse.bacc as bacc
    import concourse.mybir as mybir
    import concourse.tile as tile

    dt = mybir.dt
    nc = bacc.Bacc("TRN2", target_bir_lowering=False, debug=False)
    aps = {
        "xs": nc.dram_tensor(
            "xs", [2 * n_pairs, CIN, S], dt.float32, kind="ExternalInput"
        ).ap(),
        "cs": nc.dram_tensor(
            "cs", [2 * n_pairs, CCTX, S], dt.float32, kind="ExternalInput"
        ).ap(),
        "wqT": nc.dram_tensor(
            "wqT", [CIN, DIM], dt.bfloat16, kind="ExternalInput"
        ).ap(),
        "wkT": nc.dram_tensor(
            "wkT", [CCTX, DIM], dt.bfloat16, kind="ExternalInput"
        ).ap(),
        "wvT": nc.dram_tensor(
            "wvT", [CCTX, DIM], dt.bfloat16, kind="ExternalInput"
        ).ap(),
        "bqk": nc.dram_tensor(
            "bqk", [1, 128], dt.bfloat16, kind="ExternalInput"
        ).ap(),
        "bvp": nc.dram_tensor(
            "bvp", [1, 512], dt.bfloat16, kind="ExternalInput"
        ).ap(),
        "out": nc.dram_tensor(
            "out", [2 * n_pairs, CIN, S], dt.float32, kind="ExternalOutput"
        ).ap(),
    }
    with tile.TileContext(nc) as tc:
        build_kernel_body(nc, tc, tile, mybir, aps, n_pairs)
    nc.compile()
    return nc


def make_weight_inputs(wq, bq, wk, bk, wv, bv):
    wqT = np.ascontiguousarray((wq * SCALE).T).astype(BF16)
    wkT = np.ascontiguousarray(wk.T).astype(BF16)
    wvT = np.ascontiguousarray(wv.T).astype(BF16)
    bqk = np.concatenate([bq * SCALE, bk]).reshape(1, 128).astype(BF16)
    bvp = np.tile(bv, 8).reshape(1, 512).astype(BF16)
    return {"wqT": wqT, "wkT": wkT, "wvT": wvT, "bqk": bqk, "bvp": bvp}


_NC_CACHE = {}
_LAST_RES = None


def kernel(x, context, wq, bq, wk, bk, wv, bv):
    from concourse.bass_utils import run_bass_kernel_spmd

    x = np.asarray(x, dtype=F32)
    context = np.asarray(context, dtype=F32)

    if NPAIRS not in _NC_CACHE:
        _NC_CACHE[NPAIRS] = build_nc(NPAIRS)
    nc = _NC_CACHE[NPAIRS]

    # [B, C, D, H, W] -> [B*D, C, S]
    xr = np.ascontiguousarray(x.transpose(0, 2, 1, 3, 4)).reshape(BD, CIN, S)
    cr = np.ascontiguousarray(context.transpose(0, 2, 1, 3, 4)).reshape(
        BD, CCTX, S
    )
    winp = make_weight_inputs(
        np.asarray(wq, F32),
        np.asarray(bq, F32),
        np.asarray(wk, F32),
        np.asarray(bk, F32),
        np.asarray(wv, F32),
        np.asarray(bv, F32),
    )

    in_maps = []
    for ci in range(NCORES):
        sl = slice(ci * SLICES_PER_CORE, (ci + 1) * SLICES_PER_CORE)
        in_maps.append(
            {"xs": xr[sl], "cs": cr[sl], **winp}
        )

    res = run_bass_kernel_spmd(nc, in_maps, core_ids=list(range(NCORES)))
    global _LAST_RES
    _LAST_RES = res
    outs = [res.results[ci]["out"] for ci in range(NCORES)]
    full = np.concatenate(outs, axis=0)  # [256, 64, 4096]
    return (
        full.reshape(B, D, CIN, S)
        .transpose(0, 2, 1, 3)
        .reshape(B, CIN, D, H, W)
        .astype(F32)
    )


if __name__ == "__main__":
    # quick smoke build
    nc = build_nc(1)
    print("built OK")
